# revision 29
# baseline (speedup 1.0000x reference)
"""GsLmkEncoder Trainium2 kernel.

out[n, b*68+k] = enc_b(n,k) * exp(-0.5 * wq(n,k)),   b in 0..4
  enc_0 = dz = (x_n - l_k) . rz
  enc_1 = sin(dz), enc_2 = cos(dz), enc_3 = sin(2 dz), enc_4 = cos(2 dz)
  wq = (x_n - l_k)^T cov_k (x_n - l_k)

Reformulation: with s_n = x_n . rz and t_k = l_k . rz, dz = s_n - t_k, so
sin/cos(dz) and sin/cos(2 dz) expand by angle addition into products of
per-point trig (sin s, cos s, sin 2s, cos 2s) and per-landmark trig. wq is
quadratic in x: wq = q.u_k + x.v_k + c_k over features [x^2 terms, x, 1].

Numerics: f32r matmuls truncate operands AND products to FP22, which is
fatal for the expanded wq quadratic (|products| ~ 10^3 cancel to O(1)), so
wq uses a true-f32 matmul (4 cyc/row). The trig blocks are O(1)-magnitude
products and run in f32r at 1 cyc/row. dz is built exactly on GPSIMD from
SBUF (s - t_k) and never touches the matmul or PSUM (GPSIMD cannot read
PSUM).

Zero-row batching: 12 tiles share one stationary (12*10=120 f32 rows for
wq, 12*4=48 f32r rows for trig). The moving G for tile j is zero outside
j's row block, so one PE transpose + one ACT PSUM->SBUF copy serves 12
tiles, and the stationary base partition is always 0.

Schedule (per core, 196 tiles of 128 points):
 - bulk prologue: s = x.rz, range-reduced trig on ACT, F features built in
   a few wide DVE ops, dz_all = s - t built in chunks on GPSIMD
 - per 12 tiles: 2 transposes + 2 ACT copies; per tile: f32 wq matmul
   (68 cols) + f32r trig matmul (272 cols) into one PSUM bank
 - per 3-tile group: ACT exp -> w; DVE multiplies the 4 trig blocks by w
   (f16 out); GPSIMD multiplies dz_all * w
 - f16 output DMA (halves HBM write traffic); host casts back to f32
"""

import sys
import numpy as np

for _p in ("/opt/trn_rl_repo", "/root/.axon_site/_ro/pypackages"):
    if _p not in sys.path:
        sys.path.insert(0, _p)

import concourse.bass as bass
import concourse.bacc as bacc
import concourse.tile as tile
from concourse import mybir
from concourse.masks import make_identity
from concourse.bass_utils import run_bass_kernel_spmd

# Wire the NTFF profile hook (the agent image's antenv lacks axon_hooks);
# without it trace=True silently degrades to no profiling.
try:
    import antenv.axon_hooks  # noqa: F401
except ImportError:
    try:
        import types as _types

        sys.path.insert(0, "/root/.axon_site")
        from trn_agent_boot.trn_boot import _ntff_profile_via_ctypes

        _hook = _ntff_profile_via_ctypes("/opt/axon/libaxon_pjrt.so")
        _m = _types.ModuleType("antenv.axon_hooks")
        _m.get_axon_ntff_profile_hook = lambda: _hook
        _m.set_axon_ntff_profile_hook = lambda h: None
        sys.modules["antenv.axon_hooks"] = _m
    except Exception:
        pass

F32 = mybir.dt.float32
F32R = mybir.dt.float32r
F16 = mybir.dt.float16
AF = mybir.ActivationFunctionType
OP = mybir.AluOpType

N = 200000
L = 68
OUT_DIM = 5 * L      # 340
NCORES = 8
NTILES = 196                 # tiles of 128 points per shard
NPAD = NTILES * 128          # 25088 per shard
SUP = 12                     # tiles per stationary super-group
K1 = 10                      # wq features [x0,x1,x2,1,xx...]
K2 = 4                       # trig features [s1,c1,s2,c2]
TPG = 3                      # tiles per psum/mm group
OFL = 6                      # tiles per output DMA flush
DZC = 28                     # tiles per GPSIMD dz-build chunk
HALF_PI = float(np.pi / 2)
TWO_PI = float(np.float32(2 * np.pi))
PI_F = float(np.float32(np.pi))
INV_2PI = float(np.float32(1.0 / (2 * np.pi)))
INV_PI = float(np.float32(1.0 / np.pi))
MAGIC = 12582912.0  # 1.5 * 2**23: add+sub rounds f32 to nearest int
# clamp bounds keeping func(scale*in+bias) strictly inside [-pi, pi]
B1 = 3.141590
C1 = (-B1, B1)
C1C = (-(B1 + HALF_PI), float(np.float32(B1 - HALF_PI)))
C2 = (-B1 / 2, B1 / 2)
C2C = (-(B1 + HALF_PI) / 2, float(np.float32((B1 - HALF_PI) / 2)))


def _bcast_block(ap, nrep, block):
    """Insert a stride-0 dim of size nrep before the last dim (size block)."""
    new = ap.copy()
    pat = [list(d) for d in new.ap]
    assert pat[-1][1] == block, (pat, block)
    pat.insert(len(pat) - 1, [0, nrep])
    return bass.AP(ap.tensor, ap.offset, pat)


def _bcast_last(ap, nrep):
    """Append a stride-0 dim of size nrep after the last dim."""
    new = ap.copy()
    pat = [list(d) for d in new.ap]
    pat.append([0, nrep])
    return bass.AP(ap.tensor, ap.offset, pat)


def build_nc(pool_dz=True, ntiles=NTILES):
    npad = ntiles * 128
    nc = bacc.Bacc("TRN2", target_bir_lowering=False, debug=False, num_devices=NCORES)
    x_d = nc.dram_tensor("x", [npad, 3], F32, kind="ExternalInput")
    g1_d = nc.dram_tensor("g1", [SUP * K1, SUP * L], F32, kind="ExternalInput")
    g2_d = nc.dram_tensor("g2", [SUP * K2, SUP * 4 * L], F32R, kind="ExternalInput")
    t_d = nc.dram_tensor("tt", [128, L], F32, kind="ExternalInput")
    rz_d = nc.dram_tensor("rzb", [128, 3], F32, kind="ExternalInput")
    out_d = nc.dram_tensor("out", [npad, OUT_DIM], F16, kind="ExternalOutput")

    with tile.TileContext(nc) as tc:
        with (
            tc.tile_pool(name="const", bufs=1) as constp,
            tc.tile_pool(name="ft1pool", bufs=2) as ft1pool,
            tc.tile_pool(name="ft2pool", bufs=2) as ft2pool,
            tc.tile_pool(name="wpool", bufs=3) as wpool,
            tc.tile_pool(name="opool", bufs=3) as opool,
            tc.tile_pool(name="mmps", bufs=2, space="PSUM") as mmpsp,
            tc.tile_pool(name="ftps", bufs=1, space="PSUM") as ftpsp,
        ):
            # ---- persistent tiles ----
            x_sb = constp.tile([128, ntiles, 3], F32)
            s_all = constp.tile([128, ntiles], F32)
            ang = constp.tile([128, ntiles, 4], F32)
            scr = constp.tile([128, ntiles], F32)
            trig = constp.tile([128, ntiles, 4], F32)
            f1_all = constp.tile([128, ntiles, K1], F32)
            f2_all = constp.tile([128, ntiles, K2], F32R)
            dz_all = constp.tile([128, ntiles, L], F32)
            g1_sb = constp.tile([SUP * K1, SUP * L], F32)
            g2_sb = constp.tile([SUP * K2, SUP * 4 * L], F32R)
            t_sb = constp.tile([128, L], F32)
            rz_sb = constp.tile([128, 3], F32)
            ident = constp.tile([128, 128], F32)
            identr_t = constp.tile([128, 128], F32R)

            nc.sync.dma_start(g1_sb[:], g1_d[:])
            nc.sync.dma_start(g2_sb[:], g2_d[:])
            nc.sync.dma_start(t_sb[:], t_d[:])
            nc.sync.dma_start(rz_sb[:], rz_d[:])
            make_identity(nc, ident[:])
            bias_hpi = constp.tile([128, 1], F32)
            nc.gpsimd.memset(bias_hpi[:], HALF_PI)
            nc.gpsimd.memset(f1_all[:, :, 3:4], 1.0)  # const-1 feature
            nc.scalar.copy(identr_t[:], ident[:])

            # x load: partition p holds points p*ntiles .. p*ntiles+ntiles-1
            nc.sync.dma_start(
                x_sb[:], x_d[:].rearrange("(p m) c -> p m c", p=128)
            )

            # ---- prologue: s = x . rz, then bulk trig ----
            # absorb the x/rz DMA waits on DVE first: TensorScalarPtr
            # encodings only have one sync-wait slot
            nc.vector.tensor_tensor(
                scr[:, 0:3], x_sb[:, 0, :], rz_sb[:, 0:3], OP.mult
            )
            nc.vector.tensor_scalar(
                s_all[:], x_sb[:, :, 0], rz_sb[:, 0:1], None, OP.mult
            )
            nc.vector.scalar_tensor_tensor(
                s_all[:], x_sb[:, :, 1], rz_sb[:, 1:2], s_all[:], OP.mult, OP.add
            )
            nc.vector.scalar_tensor_tensor(
                s_all[:], x_sb[:, :, 2], rz_sb[:, 2:3], s_all[:], OP.mult, OP.add
            )
            # range-reduce the four angle families into [-pi, pi] after
            # the activation's own scale/bias is applied
            fams = [
                (INV_2PI, 0.0, -TWO_PI, C1),     # sin(s)
                (INV_2PI, 0.25, -TWO_PI, C1C),   # sin(s + pi/2)
                (INV_PI, 0.0, -PI_F, C2),        # sin(2s)
                (INV_PI, 0.25, -PI_F, C2C),      # sin(2s + pi/2)
            ]
            for ci, (inv, delta, mul, (lo, hi)) in enumerate(fams):
                # n = round(s*inv + delta) via the 1.5*2^23 magic constant;
                # delta must be added before the magic (ULP there is 1.0)
                nc.vector.tensor_scalar(
                    scr[:], s_all[:], inv, delta, OP.mult, OP.add
                )
                nc.vector.tensor_scalar(
                    scr[:], scr[:], MAGIC, MAGIC, OP.add, OP.subtract
                )
                nc.vector.scalar_tensor_tensor(
                    scr[:], scr[:], mul, s_all[:], OP.mult, OP.add
                )
                nc.vector.tensor_scalar(
                    ang[:, :, ci], scr[:], hi, lo, OP.min, OP.max
                )
            nc.scalar.activation(trig[:, :, 0], ang[:, :, 0], AF.Sin)
            nc.scalar.activation(trig[:, :, 1], ang[:, :, 1], AF.Sin, bias=bias_hpi[:])
            nc.scalar.activation(trig[:, :, 2], ang[:, :, 2], AF.Sin, scale=2.0)
            nc.scalar.activation(
                trig[:, :, 3], ang[:, :, 3], AF.Sin, bias=bias_hpi[:], scale=2.0
            )

            # ---- bulk F build ----
            # f1 rows: [x0,x1,x2, 1, x0^2,x1^2,x2^2, x0x1,x0x2,x1x2] (f32)
            nc.vector.tensor_copy(f1_all[:, :, 0:3], x_sb[:])
            nc.vector.tensor_tensor(
                f1_all[:, :, 4:7], x_sb[:], x_sb[:], OP.mult
            )
            nc.vector.tensor_tensor(
                f1_all[:, :, 7:8], x_sb[:, :, 0:1], x_sb[:, :, 1:2], OP.mult
            )
            nc.vector.tensor_tensor(
                f1_all[:, :, 8:9], x_sb[:, :, 0:1], x_sb[:, :, 2:3], OP.mult
            )
            nc.vector.tensor_tensor(
                f1_all[:, :, 9:10], x_sb[:, :, 1:2], x_sb[:, :, 2:3], OP.mult
            )
            nc.vector.tensor_copy(f2_all[:, :, 0:4], trig[:])

            # dz_all = s - t_k, in chunks on GPSIMD (fills Pool idle time)
            dz_eng = nc.gpsimd if pool_dz else nc.vector
            for c0 in range(0, ntiles, DZC):
                c1_ = min(c0 + DZC, ntiles)
                dz_eng.tensor_tensor(
                    dz_all[:, c0:c1_, :],
                    _bcast_last(s_all[:, c0:c1_], L),
                    _bcast_block(t_sb[:], c1_ - c0, L),
                    OP.subtract,
                )

            out_rows = out_d[:].rearrange("(p m) c -> p (m c)", p=128)

            # ---- main loop ----
            sup_tiles = {}
            next_sup = 0

            def emit_super(s):
                ns = min(SUP, ntiles - s * SUP)
                nr1, nr2 = ns * K1, ns * K2
                ft1_ps = ftpsp.tile([128, 128], F32, tag="FT1")
                nc.tensor.matmul(
                    ft1_ps[0:nr1, 0:128],
                    f1_all[:, s * SUP : s * SUP + ns, :].rearrange(
                        "p t k -> p (t k)"
                    ),
                    ident[:],
                    is_transpose=True,
                )
                ft1_sb = ft1pool.tile([128, 128], F32, tag="FT1S")
                nc.scalar.copy(ft1_sb[0:nr1, :], ft1_ps[0:nr1, :])
                ft2_ps = ftpsp.tile([64, 128], F32R, tag="FT2")
                nc.tensor.matmul(
                    ft2_ps[0:nr2, 0:128],
                    f2_all[:, s * SUP : s * SUP + ns, :].rearrange(
                        "p t k -> p (t k)"
                    ),
                    identr_t[:],
                    is_transpose=True,
                )
                ft2_sb = ft2pool.tile([64, 128], F32R, tag="FT2S")
                nc.scalar.copy(ft2_sb[0:nr2, :], ft2_ps[0:nr2, :])
                sup_tiles[s] = (ft1_sb, ft2_sb, nr1, nr2)

            n_fl = (ntiles + OFL - 1) // OFL
            for fb in range(n_fl):
                nt_f = min(OFL, ntiles - fb * OFL)
                o_t = opool.tile([128, OFL * OUT_DIM], F16, tag="O")
                off = 0
                while off < nt_f:
                    tpg = min(TPG, nt_f - off)
                    j0 = fb * OFL + off
                    while next_sup * SUP < j0 + tpg:
                        emit_super(next_sup)
                        next_sup += 1
                    psum = mmpsp.tile([128, TPG, 512], F32, tag="P")
                    for jj in range(tpg):
                        j = j0 + jj
                        sj = j % SUP
                        ft1_sb, ft2_sb, nr1, nr2 = sup_tiles[j // SUP]
                        nc.tensor.matmul(
                            psum[:, jj, 0:L],
                            ft1_sb[0:nr1, 0:128],
                            g1_sb[0:nr1, sj * L : (sj + 1) * L],
                            start=True,
                            stop=True,
                        )
                        nc.tensor.matmul(
                            psum[:, jj, L : 5 * L],
                            ft2_sb[0:nr2, 0:128],
                            g2_sb[0:nr2, sj * 4 * L : (sj + 1) * 4 * L],
                            start=True,
                            stop=True,
                        )
                    w_t = wpool.tile([128, TPG, L], F32, tag="W")
                    nc.scalar.activation(
                        w_t[:, 0:tpg, :], psum[:, 0:tpg, 0:L], AF.Exp, scale=-0.5
                    )
                    o4 = o_t[:, off * OUT_DIM : (off + tpg) * OUT_DIM].rearrange(
                        "p (t b l) -> p t b l", b=5, l=L
                    )
                    enc_t = psum[:, 0:tpg, L : 5 * L].rearrange(
                        "p t (b l) -> p t b l", l=L
                    )
                    nc.vector.tensor_tensor(
                        o4[:, :, 1:5, :],
                        enc_t,
                        _bcast_block(w_t[:, 0:tpg, :], 4, L),
                        OP.mult,
                    )
                    dz_eng.tensor_tensor(
                        o4[:, :, 0, :],
                        dz_all[:, j0 : j0 + tpg, :],
                        w_t[:, 0:tpg, :],
                        OP.mult,
                    )
                    off += tpg
                nc.sync.dma_start(
                    out_rows[
                        :, fb * OFL * OUT_DIM : (fb * OFL + nt_f) * OUT_DIM
                    ],
                    o_t[:, 0 : nt_f * OUT_DIM],
                )
    nc.compile()
    return nc


def host_params(l, r, scaling, rotation):
    """Zero-row-batched G1 [120, 816] f32, G2 [48, 3264] f32r + t/rz."""
    l = l.astype(np.float64)
    r = r.astype(np.float64)
    scaling = scaling.astype(np.float64)
    rotation = rotation.astype(np.float64)

    rz = r[:3, 2]
    qn = rotation / np.maximum(
        np.linalg.norm(rotation, axis=1, keepdims=True), 1e-12
    )
    w, x, y, z = qn[:, 0], qn[:, 1], qn[:, 2], qn[:, 3]
    R = np.empty((L, 3, 3), np.float64)
    R[:, 0, 0] = 1 - 2 * (y * y + z * z)
    R[:, 0, 1] = 2 * (x * y - w * z)
    R[:, 0, 2] = 2 * (x * z + w * y)
    R[:, 1, 0] = 2 * (x * y + w * z)
    R[:, 1, 1] = 1 - 2 * (x * x + z * z)
    R[:, 1, 2] = 2 * (y * z - w * x)
    R[:, 2, 0] = 2 * (x * z - w * y)
    R[:, 2, 1] = 2 * (y * z + w * x)
    R[:, 2, 2] = 1 - 2 * (x * x + y * y)
    M = R / scaling[:, None, :]
    cov = np.einsum("lij,lkj->lik", M, M)       # [L,3,3]

    b = np.einsum("lij,lj->li", cov, l)         # cov_k @ l_k
    c = np.einsum("li,li->l", l, b)             # l^T cov l
    t = l @ rz

    # G1 core [K1, L]: rows [x (-2b), 1 (c), x^2.. (u)]
    g1c = np.zeros((K1, L), np.float32)
    g1c[0:3] = -2 * b.T
    g1c[3] = c
    g1c[4:7] = np.stack([cov[:, 0, 0], cov[:, 1, 1], cov[:, 2, 2]], 0)
    g1c[7] = 2 * cov[:, 0, 1]
    g1c[8] = 2 * cov[:, 0, 2]
    g1c[9] = 2 * cov[:, 1, 2]
    # G2 core [K2, 4L]: rows [s1,c1,s2,c2]; blocks [sin|cos|sin2|cos2]
    c1, s1 = np.cos(t), np.sin(t)
    c2, s2 = np.cos(2 * t), np.sin(2 * t)
    g2c = np.zeros((K2, 4 * L), np.float32)
    g2c[0, 0:L] = c1
    g2c[1, 0:L] = -s1
    g2c[0, L : 2 * L] = s1
    g2c[1, L : 2 * L] = c1
    g2c[2, 2 * L : 3 * L] = c2
    g2c[3, 2 * L : 3 * L] = -s2
    g2c[2, 3 * L : 4 * L] = s2
    g2c[3, 3 * L : 4 * L] = c2

    G1 = np.zeros((SUP * K1, SUP * L), np.float32)
    G2 = np.zeros((SUP * K2, SUP * 4 * L), np.float32)
    for j in range(SUP):
        G1[j * K1 : (j + 1) * K1, j * L : (j + 1) * L] = g1c
        G2[j * K2 : (j + 1) * K2, j * 4 * L : (j + 1) * 4 * L] = g2c

    tt = np.broadcast_to(t.astype(np.float32), (128, L)).copy()
    rzb = np.broadcast_to(rz.astype(np.float32), (128, 3)).copy()
    return G1, G2, tt, rzb


_NC_CACHE = {}


def _get_nc(pool_dz=True):
    key = bool(pool_dz)
    if key not in _NC_CACHE:
        _NC_CACHE[key] = build_nc(pool_dz=key)
    return _NC_CACHE[key]


def run(inputs, pool_dz=True, trace=False, **_kw):
    x = inputs["x"]
    G1, G2, tt, rzb = host_params(
        inputs["l"], inputs["r"], inputs["scaling"], inputs["rotation"]
    )
    xpad = np.zeros((NCORES * NPAD, 3), np.float32)
    xpad[:N] = x
    shards = xpad.reshape(NCORES, NPAD, 3)
    in_maps = []
    for i in range(NCORES):
        m = {
            "x": np.ascontiguousarray(shards[i]),
            "g1": G1,
            "g2": G2,
            "tt": tt,
            "rzb": rzb,
        }
        in_maps.append(m)
    nc = _get_nc(pool_dz)
    res = run_bass_kernel_spmd(nc, in_maps, list(range(NCORES)), trace=trace)
    out = np.concatenate([r["out"] for r in res.results], axis=0)[:N]
    return np.ascontiguousarray(out.astype(np.float32)), res


def kernel(**inputs):
    out, _ = run(inputs)
    return out


# revision 37
# speedup vs baseline: 1.5945x; 1.5945x over previous
"""GsLmkEncoder Trainium2 kernel.

out[n, b*68+k] = enc_b(n,k) * exp(-0.5 * wq(n,k)),   b in 0..4
  enc_0 = dz = (x_n - l_k) . rz
  enc_1 = sin(dz), enc_2 = cos(dz), enc_3 = sin(2 dz), enc_4 = cos(2 dz)
  wq = (x_n - l_k)^T cov_k (x_n - l_k)

Reformulation: with s_n = x_n . rz and t_k = l_k . rz, dz = s_n - t_k, so
sin/cos(dz) and sin/cos(2 dz) expand by angle addition into products of
per-point trig (sin s, cos s, sin 2s, cos 2s) and per-landmark trig. wq is
quadratic in x: wq = q.u_k + x.v_k + c_k over features [x^2 terms, x, 1].

Numerics: f32r matmuls truncate operands AND products to FP22, which is
fatal for the expanded wq quadratic (|products| ~ 10^3 cancel to O(1)), so
wq uses a true-f32 matmul (4 cyc/row). The trig blocks are O(1)-magnitude
products and run in f32r at 1 cyc/row. dz is built exactly on GPSIMD from
SBUF (s - t_k) and never touches the matmul or PSUM (GPSIMD cannot read
PSUM).

Zero-row batching: 12 tiles share one stationary (12*10=120 f32 rows for
wq, 12*4=48 f32r rows for trig). The moving G for tile j is zero outside
j's row block, so one PE transpose + one ACT PSUM->SBUF copy serves 12
tiles, and the stationary base partition is always 0.

Schedule (per core, 196 tiles of 128 points):
 - bulk prologue: s = x.rz, range-reduced trig on ACT, F features built in
   a few wide DVE ops, dz_all = s - t built in chunks on GPSIMD
 - per 12 tiles: 2 transposes + 2 ACT copies; per tile: f32 wq matmul
   (68 cols) + f32r trig matmul (272 cols) into one PSUM bank
 - per 3-tile group: ACT exp -> w; DVE multiplies the 4 trig blocks by w
   (f16 out); GPSIMD multiplies dz_all * w
 - f16 output DMA (halves HBM write traffic); host casts back to f32
"""

import sys
import numpy as np

for _p in ("/opt/trn_rl_repo", "/root/.axon_site/_ro/pypackages"):
    if _p not in sys.path:
        sys.path.insert(0, _p)

import concourse.bass as bass
import concourse.bacc as bacc
import concourse.tile as tile
from concourse import mybir
from concourse.masks import make_identity
from concourse.bass_utils import run_bass_kernel_spmd

# Wire the NTFF profile hook (the agent image's antenv lacks axon_hooks);
# without it trace=True silently degrades to no profiling.
try:
    import antenv.axon_hooks  # noqa: F401
except ImportError:
    try:
        import types as _types

        sys.path.insert(0, "/root/.axon_site")
        from trn_agent_boot.trn_boot import _ntff_profile_via_ctypes

        _hook = _ntff_profile_via_ctypes("/opt/axon/libaxon_pjrt.so")
        _m = _types.ModuleType("antenv.axon_hooks")
        _m.get_axon_ntff_profile_hook = lambda: _hook
        _m.set_axon_ntff_profile_hook = lambda h: None
        sys.modules["antenv.axon_hooks"] = _m
    except Exception:
        pass

F32 = mybir.dt.float32
F32R = mybir.dt.float32r
F16 = mybir.dt.float16
BF16 = mybir.dt.bfloat16
AF = mybir.ActivationFunctionType
OP = mybir.AluOpType

N = 200000
L = 68
OUT_DIM = 5 * L      # 340
NCORES = 8
NTILES = 196                 # tiles of 128 points per shard
NPAD = NTILES * 128          # 25088 per shard
SUP = 6                      # tiles per stationary super-group
K1 = 10                      # wq features [x0,x1,x2,1,xx...]
K2 = 4                       # trig features [s1,c1,s2,c2]
TPG = 3                      # tiles per psum/mm group
OFL = 6                      # tiles per output DMA flush
DZC = 28                     # tiles per GPSIMD dz-build chunk
HALF_PI = float(np.pi / 2)
TWO_PI = float(np.float32(2 * np.pi))
PI_F = float(np.float32(np.pi))
INV_2PI = float(np.float32(1.0 / (2 * np.pi)))
INV_PI = float(np.float32(1.0 / np.pi))
MAGIC = 12582912.0  # 1.5 * 2**23: add+sub rounds f32 to nearest int
# clamp bounds keeping func(scale*in+bias) strictly inside [-pi, pi]
B1 = 3.141590
C1 = (-B1, B1)
C1C = (-(B1 + HALF_PI), float(np.float32(B1 - HALF_PI)))
C2 = (-B1 / 2, B1 / 2)
C2C = (-(B1 + HALF_PI) / 2, float(np.float32((B1 - HALF_PI) / 2)))


def _bcast_block(ap, nrep, block):
    """Insert a stride-0 dim of size nrep before the last dim (size block)."""
    new = ap.copy()
    pat = [list(d) for d in new.ap]
    assert pat[-1][1] == block, (pat, block)
    pat.insert(len(pat) - 1, [0, nrep])
    return bass.AP(ap.tensor, ap.offset, pat)


def _bcast_last(ap, nrep):
    """Append a stride-0 dim of size nrep after the last dim."""
    new = ap.copy()
    pat = [list(d) for d in new.ap]
    pat.append([0, nrep])
    return bass.AP(ap.tensor, ap.offset, pat)


def build_nc(pool_dz=True, ntiles=NTILES):
    npad = ntiles * 128
    nc = bacc.Bacc("TRN2", target_bir_lowering=False, debug=False, num_devices=NCORES)
    x_d = nc.dram_tensor("x", [npad, 3], F32, kind="ExternalInput")
    g1_d = nc.dram_tensor("g1", [SUP * K1, SUP * L], F32, kind="ExternalInput")
    g2_d = nc.dram_tensor("g2", [SUP * K2, SUP * 4 * L], BF16, kind="ExternalInput")
    t_d = nc.dram_tensor("tt", [128, L], F32, kind="ExternalInput")
    rz_d = nc.dram_tensor("rzb", [128, 3], F32, kind="ExternalInput")
    out_d = nc.dram_tensor("out", [npad, OUT_DIM], F16, kind="ExternalOutput")

    with tile.TileContext(nc) as tc:
        with (
            tc.tile_pool(name="const", bufs=1) as constp,
            tc.tile_pool(name="ft1pool", bufs=2) as ft1pool,
            tc.tile_pool(name="ft2pool", bufs=2) as ft2pool,
            tc.tile_pool(name="wpool", bufs=3) as wpool,
            tc.tile_pool(name="opool", bufs=3) as opool,
            tc.tile_pool(name="mmps", bufs=2, space="PSUM") as mmpsp,
            tc.tile_pool(name="wqps", bufs=1, space="PSUM") as wqpsp,
            tc.tile_pool(name="ftps", bufs=1, space="PSUM") as ftpsp,
        ):
            # ---- persistent tiles ----
            x_sb = constp.tile([128, ntiles, 3], F32)
            s_all = constp.tile([128, ntiles], F32)
            ang = constp.tile([128, ntiles, 4], F32)
            scr = constp.tile([128, ntiles], F32)
            trig = constp.tile([128, ntiles, 4], F32)
            f1_all = constp.tile([128, ntiles, K1], F32)
            f2_all = constp.tile([128, ntiles, K2], F32)
            dz_all = constp.tile([128, ntiles, L], F32)
            g1_sb = constp.tile([SUP * K1, SUP * L], F32)
            g2_sb = constp.tile([SUP * K2, SUP * 4 * L], BF16)
            t_sb = constp.tile([128, L], F32)
            rz_sb = constp.tile([128, 3], F32)
            ident = constp.tile([128, 128], F32)

            nc.sync.dma_start(g1_sb[:], g1_d[:])
            nc.sync.dma_start(g2_sb[:], g2_d[:])
            nc.sync.dma_start(t_sb[:], t_d[:])
            nc.sync.dma_start(rz_sb[:], rz_d[:])
            make_identity(nc, ident[:])
            bias_hpi = constp.tile([128, 1], F32)
            nc.gpsimd.memset(bias_hpi[:], HALF_PI)
            nc.gpsimd.memset(f1_all[:, :, 3:4], 1.0)  # const-1 feature

            # x load: partition p holds points p*ntiles .. p*ntiles+ntiles-1
            nc.sync.dma_start(
                x_sb[:], x_d[:].rearrange("(p m) c -> p m c", p=128)
            )

            # ---- prologue: s = x . rz, then bulk trig ----
            # absorb the x/rz DMA waits on DVE first: TensorScalarPtr
            # encodings only have one sync-wait slot
            nc.vector.tensor_tensor(
                scr[:, 0:3], x_sb[:, 0, :], rz_sb[:, 0:3], OP.mult
            )
            nc.vector.tensor_scalar(
                s_all[:], x_sb[:, :, 0], rz_sb[:, 0:1], None, OP.mult
            )
            nc.vector.scalar_tensor_tensor(
                s_all[:], x_sb[:, :, 1], rz_sb[:, 1:2], s_all[:], OP.mult, OP.add
            )
            nc.vector.scalar_tensor_tensor(
                s_all[:], x_sb[:, :, 2], rz_sb[:, 2:3], s_all[:], OP.mult, OP.add
            )
            # range-reduce the four angle families into [-pi, pi] after
            # the activation's own scale/bias is applied
            fams = [
                (INV_2PI, 0.0, -TWO_PI, C1),     # sin(s)
                (INV_2PI, 0.25, -TWO_PI, C1C),   # sin(s + pi/2)
                (INV_PI, 0.0, -PI_F, C2),        # sin(2s)
                (INV_PI, 0.25, -PI_F, C2C),      # sin(2s + pi/2)
            ]
            for ci, (inv, delta, mul, (lo, hi)) in enumerate(fams):
                # n = round(s*inv + delta) via the 1.5*2^23 magic constant;
                # delta must be added before the magic (ULP there is 1.0)
                nc.vector.tensor_scalar(
                    scr[:], s_all[:], inv, delta, OP.mult, OP.add
                )
                nc.vector.tensor_scalar(
                    scr[:], scr[:], MAGIC, MAGIC, OP.add, OP.subtract
                )
                nc.vector.scalar_tensor_tensor(
                    scr[:], scr[:], mul, s_all[:], OP.mult, OP.add
                )
                nc.vector.tensor_scalar(
                    ang[:, :, ci], scr[:], hi, lo, OP.min, OP.max
                )
            nc.scalar.activation(trig[:, :, 0], ang[:, :, 0], AF.Sin)
            nc.scalar.activation(trig[:, :, 1], ang[:, :, 1], AF.Sin, bias=bias_hpi[:])
            nc.scalar.activation(trig[:, :, 2], ang[:, :, 2], AF.Sin, scale=2.0)
            nc.scalar.activation(
                trig[:, :, 3], ang[:, :, 3], AF.Sin, bias=bias_hpi[:], scale=2.0
            )

            # ---- bulk F build ----
            # f1 rows: [x0,x1,x2, 1, x0^2,x1^2,x2^2, x0x1,x0x2,x1x2] (f32)
            nc.vector.tensor_copy(f1_all[:, :, 0:3], x_sb[:])
            nc.vector.tensor_tensor(
                f1_all[:, :, 4:7], x_sb[:], x_sb[:], OP.mult
            )
            nc.vector.tensor_tensor(
                f1_all[:, :, 7:8], x_sb[:, :, 0:1], x_sb[:, :, 1:2], OP.mult
            )
            nc.vector.tensor_tensor(
                f1_all[:, :, 8:9], x_sb[:, :, 0:1], x_sb[:, :, 2:3], OP.mult
            )
            nc.vector.tensor_tensor(
                f1_all[:, :, 9:10], x_sb[:, :, 1:2], x_sb[:, :, 2:3], OP.mult
            )
            nc.vector.tensor_copy(f2_all[:, :, 0:4], trig[:])

            # dz_all = s - t_k, in chunks on GPSIMD (fills Pool idle time)
            dz_eng = nc.gpsimd if pool_dz else nc.vector
            for c0 in range(0, ntiles, DZC):
                c1_ = min(c0 + DZC, ntiles)
                dz_eng.tensor_tensor(
                    dz_all[:, c0:c1_, :],
                    _bcast_last(s_all[:, c0:c1_], L),
                    _bcast_block(t_sb[:], c1_ - c0, L),
                    OP.subtract,
                )

            out_rows = out_d[:].rearrange("(p m) c -> p (m c)", p=128)

            # ---- main loop ----
            # super-group s (6 tiles): one transposed stationary pair
            # (ft1 [60,128] f32 for wq, ft2 [24,128] bf16 for trig), one
            # 408-col f32 wq matmul, per-tile 272-col bf16 trig matmuls.
            sup_tiles = {}
            n_sup = (ntiles + SUP - 1) // SUP

            def emit_transposes(s):
                ns = min(SUP, ntiles - s * SUP)
                nr1, nr2 = ns * K1, ns * K2
                ft_ps = ftpsp.tile([128, 2, 128], F32, tag="FT")
                nc.tensor.matmul(
                    ft_ps[0:nr1, 0, :],
                    f1_all[:, s * SUP : s * SUP + ns, :].rearrange(
                        "p t k -> p (t k)"
                    ),
                    ident[:],
                    is_transpose=True,
                    skip_group_check=True,
                )
                nc.tensor.matmul(
                    ft_ps[0:nr2, 1, :],
                    f2_all[:, s * SUP : s * SUP + ns, :].rearrange(
                        "p t k -> p (t k)"
                    ),
                    ident[:],
                    is_transpose=True,
                    skip_group_check=True,
                )
                ft1_sb = ft1pool.tile([128, 128], F32, tag="FT1S")
                nc.scalar.copy(ft1_sb[0:nr1, :], ft_ps[0:nr1, 0, :])
                ft2_sb = ft2pool.tile([64, 128], BF16, tag="FT2S")
                nc.scalar.copy(ft2_sb[0:nr2, :], ft_ps[0:nr2, 1, :])
                sup_tiles[s] = (ft1_sb, ft2_sb, nr1, nr2)

            emit_transposes(0)
            wq_ps = None
            n_fl = (ntiles + OFL - 1) // OFL
            for fb in range(n_fl):
                nt_f = min(OFL, ntiles - fb * OFL)
                o_t = opool.tile([128, OFL * OUT_DIM], F16, tag="O")
                off = 0
                while off < nt_f:
                    tpg = min(TPG, nt_f - off)
                    j0 = fb * OFL + off
                    g = j0 // TPG
                    s = j0 // SUP
                    sj0 = j0 % SUP
                    ft1_sb, ft2_sb, nr1, nr2 = sup_tiles[s]
                    if sj0 == 0:
                        # first group of a super: wq matmul for all its tiles
                        ns = min(SUP, ntiles - s * SUP)
                        wq_ps = wqpsp.tile([128, 512], F32, tag="WQ")
                        nc.tensor.matmul(
                            wq_ps[:, 0 : ns * L],
                            ft1_sb[0:nr1, 0:128],
                            g1_sb[0:nr1, 0 : ns * L],
                            start=True,
                            stop=True,
                        )
                    psum = mmpsp.tile([128, TPG, 512], F32, tag="P")
                    for jj in range(tpg):
                        j = j0 + jj
                        sj = j % SUP
                        nc.tensor.matmul(
                            psum[:, jj, 0 : 4 * L],
                            ft2_sb[0:nr2, 0:128],
                            g2_sb[0:nr2, sj * 4 * L : (sj + 1) * 4 * L],
                            start=True,
                            stop=True,
                        )
                    if (g % 2 == 1) and (s + 1 < n_sup) and (s + 1) not in sup_tiles:
                        emit_transposes(s + 1)
                    w_t = wpool.tile([128, TPG, L], F32, tag="W")
                    nc.scalar.activation(
                        w_t[:, 0:tpg, :],
                        wq_ps[:, sj0 * L : (sj0 + tpg) * L].rearrange(
                            "p (t l) -> p t l", l=L
                        ),
                        AF.Exp,
                        scale=-0.5,
                    )
                    o4 = o_t[:, off * OUT_DIM : (off + tpg) * OUT_DIM].rearrange(
                        "p (t b l) -> p t b l", b=5, l=L
                    )
                    enc_t = psum[:, 0:tpg, 0 : 4 * L].rearrange(
                        "p t (b l) -> p t b l", l=L
                    )
                    nc.vector.tensor_tensor(
                        o4[:, :, 1:5, :],
                        enc_t,
                        _bcast_block(w_t[:, 0:tpg, :], 4, L),
                        OP.mult,
                    )
                    dz_eng.tensor_tensor(
                        o4[:, :, 0, :],
                        dz_all[:, j0 : j0 + tpg, :],
                        w_t[:, 0:tpg, :],
                        OP.mult,
                    )
                    off += tpg
                nc.sync.dma_start(
                    out_rows[
                        :, fb * OFL * OUT_DIM : (fb * OFL + nt_f) * OUT_DIM
                    ],
                    o_t[:, 0 : nt_f * OUT_DIM],
                )
    nc.compile()
    return nc


def host_params(l, r, scaling, rotation):
    """Zero-row-batched G1 [120, 816] f32, G2 [48, 3264] f32r + t/rz."""
    l = l.astype(np.float64)
    r = r.astype(np.float64)
    scaling = scaling.astype(np.float64)
    rotation = rotation.astype(np.float64)

    rz = r[:3, 2]
    qn = rotation / np.maximum(
        np.linalg.norm(rotation, axis=1, keepdims=True), 1e-12
    )
    w, x, y, z = qn[:, 0], qn[:, 1], qn[:, 2], qn[:, 3]
    R = np.empty((L, 3, 3), np.float64)
    R[:, 0, 0] = 1 - 2 * (y * y + z * z)
    R[:, 0, 1] = 2 * (x * y - w * z)
    R[:, 0, 2] = 2 * (x * z + w * y)
    R[:, 1, 0] = 2 * (x * y + w * z)
    R[:, 1, 1] = 1 - 2 * (x * x + z * z)
    R[:, 1, 2] = 2 * (y * z - w * x)
    R[:, 2, 0] = 2 * (x * z - w * y)
    R[:, 2, 1] = 2 * (y * z + w * x)
    R[:, 2, 2] = 1 - 2 * (x * x + y * y)
    M = R / scaling[:, None, :]
    cov = np.einsum("lij,lkj->lik", M, M)       # [L,3,3]

    b = np.einsum("lij,lj->li", cov, l)         # cov_k @ l_k
    c = np.einsum("li,li->l", l, b)             # l^T cov l
    t = l @ rz

    # G1 core [K1, L]: rows [x (-2b), 1 (c), x^2.. (u)]
    g1c = np.zeros((K1, L), np.float32)
    g1c[0:3] = -2 * b.T
    g1c[3] = c
    g1c[4:7] = np.stack([cov[:, 0, 0], cov[:, 1, 1], cov[:, 2, 2]], 0)
    g1c[7] = 2 * cov[:, 0, 1]
    g1c[8] = 2 * cov[:, 0, 2]
    g1c[9] = 2 * cov[:, 1, 2]
    # G2 core [K2, 4L]: rows [s1,c1,s2,c2]; blocks [sin|cos|sin2|cos2]
    c1, s1 = np.cos(t), np.sin(t)
    c2, s2 = np.cos(2 * t), np.sin(2 * t)
    g2c = np.zeros((K2, 4 * L), np.float32)
    g2c[0, 0:L] = c1
    g2c[1, 0:L] = -s1
    g2c[0, L : 2 * L] = s1
    g2c[1, L : 2 * L] = c1
    g2c[2, 2 * L : 3 * L] = c2
    g2c[3, 2 * L : 3 * L] = -s2
    g2c[2, 3 * L : 4 * L] = s2
    g2c[3, 3 * L : 4 * L] = c2

    G1 = np.zeros((SUP * K1, SUP * L), np.float32)
    G2 = np.zeros((SUP * K2, SUP * 4 * L), np.float32)
    for j in range(SUP):
        G1[j * K1 : (j + 1) * K1, j * L : (j + 1) * L] = g1c
        G2[j * K2 : (j + 1) * K2, j * 4 * L : (j + 1) * 4 * L] = g2c

    import ml_dtypes

    G2 = G2.astype(ml_dtypes.bfloat16)
    tt = np.broadcast_to(t.astype(np.float32), (128, L)).copy()
    rzb = np.broadcast_to(rz.astype(np.float32), (128, 3)).copy()
    return G1, G2, tt, rzb


_NC_CACHE = {}


def _get_nc(pool_dz=True):
    key = bool(pool_dz)
    if key not in _NC_CACHE:
        _NC_CACHE[key] = build_nc(pool_dz=key)
    return _NC_CACHE[key]


def run(inputs, pool_dz=True, trace=False, **_kw):
    x = inputs["x"]
    G1, G2, tt, rzb = host_params(
        inputs["l"], inputs["r"], inputs["scaling"], inputs["rotation"]
    )
    xpad = np.zeros((NCORES * NPAD, 3), np.float32)
    xpad[:N] = x
    shards = xpad.reshape(NCORES, NPAD, 3)
    in_maps = []
    for i in range(NCORES):
        m = {
            "x": np.ascontiguousarray(shards[i]),
            "g1": G1,
            "g2": G2,
            "tt": tt,
            "rzb": rzb,
        }
        in_maps.append(m)
    nc = _get_nc(pool_dz)
    res = run_bass_kernel_spmd(nc, in_maps, list(range(NCORES)), trace=trace)
    out = np.concatenate([r["out"] for r in res.results], axis=0)[:N]
    return np.ascontiguousarray(out.astype(np.float32)), res


def kernel(**inputs):
    out, _ = run(inputs)
    return out


# revision 42
# speedup vs baseline: 1.6706x; 1.0477x over previous
"""GsLmkEncoder Trainium2 kernel.

out[n, b*68+k] = enc_b(n,k) * exp(-0.5 * wq(n,k)),   b in 0..4
  enc_0 = dz = (x_n - l_k) . rz
  enc_1 = sin(dz), enc_2 = cos(dz), enc_3 = sin(2 dz), enc_4 = cos(2 dz)
  wq = (x_n - l_k)^T cov_k (x_n - l_k)

Reformulation: with s_n = x_n . rz and t_k = l_k . rz, dz = s_n - t_k, so
sin/cos(dz) and sin/cos(2 dz) expand by angle addition into products of
per-point trig (sin s, cos s, sin 2s, cos 2s) and per-landmark trig. wq is
quadratic in x: wq = q.u_k + x.v_k + c_k over features [x^2 terms, x, 1].

Numerics: f32r matmuls truncate operands AND products to FP22, which is
fatal for the expanded wq quadratic (|products| ~ 10^3 cancel to O(1)), so
wq uses a true-f32 matmul (4 cyc/row). The trig blocks are O(1)-magnitude
products and run in f32r at 1 cyc/row. dz is built exactly on GPSIMD from
SBUF (s - t_k) and never touches the matmul or PSUM (GPSIMD cannot read
PSUM).

Zero-row batching: 12 tiles share one stationary (12*10=120 f32 rows for
wq, 12*4=48 f32r rows for trig). The moving G for tile j is zero outside
j's row block, so one PE transpose + one ACT PSUM->SBUF copy serves 12
tiles, and the stationary base partition is always 0.

Schedule (per core, 196 tiles of 128 points):
 - bulk prologue: s = x.rz, range-reduced trig on ACT, F features built in
   a few wide DVE ops, dz_all = s - t built in chunks on GPSIMD
 - per 12 tiles: 2 transposes + 2 ACT copies; per tile: f32 wq matmul
   (68 cols) + f32r trig matmul (272 cols) into one PSUM bank
 - per 3-tile group: ACT exp -> w; DVE multiplies the 4 trig blocks by w
   (f16 out); GPSIMD multiplies dz_all * w
 - f16 output DMA (halves HBM write traffic); host casts back to f32
"""

import sys
import numpy as np

for _p in ("/opt/trn_rl_repo", "/root/.axon_site/_ro/pypackages"):
    if _p not in sys.path:
        sys.path.insert(0, _p)

import concourse.bass as bass
import concourse.bacc as bacc
import concourse.tile as tile
from concourse import mybir
from concourse.masks import make_identity
from concourse.bass_utils import run_bass_kernel_spmd

# Wire the NTFF profile hook (the agent image's antenv lacks axon_hooks);
# without it trace=True silently degrades to no profiling.
try:
    import antenv.axon_hooks  # noqa: F401
except ImportError:
    try:
        import types as _types

        sys.path.insert(0, "/root/.axon_site")
        from trn_agent_boot.trn_boot import _ntff_profile_via_ctypes

        _hook = _ntff_profile_via_ctypes("/opt/axon/libaxon_pjrt.so")
        _m = _types.ModuleType("antenv.axon_hooks")
        _m.get_axon_ntff_profile_hook = lambda: _hook
        _m.set_axon_ntff_profile_hook = lambda h: None
        sys.modules["antenv.axon_hooks"] = _m
    except Exception:
        pass

F32 = mybir.dt.float32
F32R = mybir.dt.float32r
F16 = mybir.dt.float16
BF16 = mybir.dt.bfloat16
AF = mybir.ActivationFunctionType
OP = mybir.AluOpType

N = 200000
L = 68
OUT_DIM = 5 * L      # 340
NCORES = 8
NTILES = 196                 # tiles of 128 points per shard
NPAD = NTILES * 128          # 25088 per shard
SUP = 6                      # tiles per stationary super-group
K1 = 10                      # wq features [x0,x1,x2,1,xx...]
K2 = 4                       # trig features [s1,c1,s2,c2]
TPG = 2                      # tiles per psum/mm group
OFL = 6                      # tiles per output DMA flush
DZC = 12                     # tiles per GPSIMD dz-build chunk
HALF_PI = float(np.pi / 2)
TWO_PI = float(np.float32(2 * np.pi))
PI_F = float(np.float32(np.pi))
INV_2PI = float(np.float32(1.0 / (2 * np.pi)))
INV_PI = float(np.float32(1.0 / np.pi))
MAGIC = 12582912.0  # 1.5 * 2**23: add+sub rounds f32 to nearest int
# clamp bounds keeping func(scale*in+bias) strictly inside [-pi, pi]
B1 = 3.141590
C1 = (-B1, B1)
C1C = (-(B1 + HALF_PI), float(np.float32(B1 - HALF_PI)))
C2 = (-B1 / 2, B1 / 2)
C2C = (-(B1 + HALF_PI) / 2, float(np.float32((B1 - HALF_PI) / 2)))


def _bcast_block(ap, nrep, block):
    """Insert a stride-0 dim of size nrep before the last dim (size block)."""
    new = ap.copy()
    pat = [list(d) for d in new.ap]
    assert pat[-1][1] == block, (pat, block)
    pat.insert(len(pat) - 1, [0, nrep])
    return bass.AP(ap.tensor, ap.offset, pat)


def _bcast_last(ap, nrep):
    """Append a stride-0 dim of size nrep after the last dim."""
    new = ap.copy()
    pat = [list(d) for d in new.ap]
    pat.append([0, nrep])
    return bass.AP(ap.tensor, ap.offset, pat)


def build_nc(pool_dz=True, ntiles=NTILES):
    npad = ntiles * 128
    nc = bacc.Bacc("TRN2", target_bir_lowering=False, debug=False, num_devices=NCORES)
    x_d = nc.dram_tensor("x", [npad, 3], F32, kind="ExternalInput")
    g1_d = nc.dram_tensor("g1", [SUP * K1, SUP * L], F32, kind="ExternalInput")
    g2_d = nc.dram_tensor("g2", [SUP * K2, SUP * 4 * L], BF16, kind="ExternalInput")
    t_d = nc.dram_tensor("tt", [128, L], F32, kind="ExternalInput")
    rz_d = nc.dram_tensor("rzb", [128, 3], F32, kind="ExternalInput")
    out_d = nc.dram_tensor("out", [npad, OUT_DIM], F16, kind="ExternalOutput")

    with tile.TileContext(nc) as tc:
        with (
            tc.tile_pool(name="const", bufs=1) as constp,
            tc.tile_pool(name="ft1pool", bufs=2) as ft1pool,
            tc.tile_pool(name="ft2pool", bufs=2) as ft2pool,
            tc.tile_pool(name="wpool", bufs=3) as wpool,
            tc.tile_pool(name="opool", bufs=3) as opool,
            tc.tile_pool(name="mmps", bufs=3, space="PSUM") as mmpsp,
            tc.tile_pool(name="wqps", bufs=2, space="PSUM") as wqpsp,
        ):
            # ---- persistent tiles ----
            x_sb = constp.tile([128, ntiles, 3], F32)
            s_all = constp.tile([128, ntiles], F32)
            ang = constp.tile([128, ntiles, 4], F32)
            scr = constp.tile([128, ntiles], F32)
            trig = constp.tile([128, ntiles, 4], F32)
            f1_all = constp.tile([128, ntiles, K1], F32)
            f2_all = constp.tile([128, ntiles, K2], F32)
            dz_all = constp.tile([128, ntiles, L], F32)
            g1_sb = constp.tile([SUP * K1, SUP * L], F32)
            g2_sb = constp.tile([SUP * K2, SUP * 4 * L], BF16)
            t_sb = constp.tile([128, L], F32)
            rz_sb = constp.tile([128, 3], F32)
            ident = constp.tile([128, 128], F32)

            # x load first: partition p holds points p*ntiles .. +ntiles-1
            nc.sync.dma_start(
                x_sb[:], x_d[:].rearrange("(p m) c -> p m c", p=128)
            )
            nc.sync.dma_start(rz_sb[:], rz_d[:])
            nc.sync.dma_start(g1_sb[:], g1_d[:])
            nc.sync.dma_start(g2_sb[:], g2_d[:])
            nc.sync.dma_start(t_sb[:], t_d[:])
            make_identity(nc, ident[:])
            bias_hpi = constp.tile([128, 1], F32)
            nc.gpsimd.memset(bias_hpi[:], HALF_PI)
            nc.gpsimd.memset(f1_all[:, :, 3:4], 1.0)  # const-1 feature

            # ---- prologue: s = x . rz, then bulk trig ----
            # absorb the x/rz DMA waits on DVE first: TensorScalarPtr
            # encodings only have one sync-wait slot
            nc.vector.tensor_tensor(
                scr[:, 0:3], x_sb[:, 0, :], rz_sb[:, 0:3], OP.mult
            )
            nc.vector.tensor_scalar(
                s_all[:], x_sb[:, :, 0], rz_sb[:, 0:1], None, OP.mult
            )
            nc.vector.scalar_tensor_tensor(
                s_all[:], x_sb[:, :, 1], rz_sb[:, 1:2], s_all[:], OP.mult, OP.add
            )
            nc.vector.scalar_tensor_tensor(
                s_all[:], x_sb[:, :, 2], rz_sb[:, 2:3], s_all[:], OP.mult, OP.add
            )
            # F1 (wq features) needs only x — build before the trig fams so
            # the first transpose/wq-matmul can start early.
            nc.vector.tensor_copy(f1_all[:, :, 0:3], x_sb[:])
            nc.vector.tensor_tensor(
                f1_all[:, :, 4:7], x_sb[:], x_sb[:], OP.mult
            )
            nc.vector.tensor_tensor(
                f1_all[:, :, 7:8], x_sb[:, :, 0:1], x_sb[:, :, 1:2], OP.mult
            )
            nc.vector.tensor_tensor(
                f1_all[:, :, 8:9], x_sb[:, :, 0:1], x_sb[:, :, 2:3], OP.mult
            )
            nc.vector.tensor_tensor(
                f1_all[:, :, 9:10], x_sb[:, :, 1:2], x_sb[:, :, 2:3], OP.mult
            )

            # range-reduce the four angle families into [-pi, pi] after
            # the activation's own scale/bias is applied
            fams = [
                (INV_2PI, 0.0, -TWO_PI, C1),     # sin(s)
                (INV_2PI, 0.25, -TWO_PI, C1C),   # sin(s + pi/2)
                (INV_PI, 0.0, -PI_F, C2),        # sin(2s)
                (INV_PI, 0.25, -PI_F, C2C),      # sin(2s + pi/2)
            ]
            for ci, (inv, delta, mul, (lo, hi)) in enumerate(fams):
                # n = round(s*inv + delta) via the 1.5*2^23 magic constant;
                # delta must be added before the magic (ULP there is 1.0)
                nc.vector.tensor_scalar(
                    scr[:], s_all[:], inv, delta, OP.mult, OP.add
                )
                nc.vector.tensor_scalar(
                    scr[:], scr[:], MAGIC, MAGIC, OP.add, OP.subtract
                )
                nc.vector.scalar_tensor_tensor(
                    scr[:], scr[:], mul, s_all[:], OP.mult, OP.add
                )
                nc.vector.tensor_scalar(
                    ang[:, :, ci], scr[:], hi, lo, OP.min, OP.max
                )
            nc.scalar.activation(trig[:, :, 0], ang[:, :, 0], AF.Sin)
            nc.scalar.activation(trig[:, :, 1], ang[:, :, 1], AF.Sin, bias=bias_hpi[:])
            nc.scalar.activation(trig[:, :, 2], ang[:, :, 2], AF.Sin, scale=2.0)
            nc.scalar.activation(
                trig[:, :, 3], ang[:, :, 3], AF.Sin, bias=bias_hpi[:], scale=2.0
            )

            nc.vector.tensor_copy(f2_all[:, :, 0:4], trig[:])

            dz_eng = nc.gpsimd if pool_dz else nc.vector
            next_dzc = [0]

            def emit_dz_chunks(upto):
                # dz_all = s - t_k; GPSIMD is in-order, so chunks are
                # emitted lazily just ahead of their consumers
                while next_dzc[0] < min(upto, ntiles):
                    c0 = next_dzc[0]
                    c1_ = min(c0 + DZC, ntiles)
                    dz_eng.tensor_tensor(
                        dz_all[:, c0:c1_, :],
                        _bcast_last(s_all[:, c0:c1_], L),
                        _bcast_block(t_sb[:], c1_ - c0, L),
                        OP.subtract,
                    )
                    next_dzc[0] = c1_

            out_rows = out_d[:].rearrange("(p m) c -> p (m c)", p=128)

            # ---- main loop ----
            # super-group s (6 tiles): one transposed stationary pair
            # (ft1 [60,128] f32 for wq, ft2 [24,128] bf16 for trig), one
            # 408-col f32 wq matmul, per-tile 272-col bf16 trig matmuls.
            # Transposes ride in the unused tails of trig PSUM banks.
            sup_tiles = {}
            n_sup = (ntiles + SUP - 1) // SUP

            def emit_transposes(s, ps):
                ns = min(SUP, ntiles - s * SUP)
                nr1, nr2 = ns * K1, ns * K2
                nc.tensor.matmul(
                    ps[0:nr1, 0, 4 * L : 4 * L + 128],
                    f1_all[:, s * SUP : s * SUP + ns, :].rearrange(
                        "p t k -> p (t k)"
                    ),
                    ident[:],
                    is_transpose=True,
                    skip_group_check=True,
                )
                nc.tensor.matmul(
                    ps[0:nr2, 1, 4 * L : 4 * L + 128],
                    f2_all[:, s * SUP : s * SUP + ns, :].rearrange(
                        "p t k -> p (t k)"
                    ),
                    ident[:],
                    is_transpose=True,
                    skip_group_check=True,
                )
                ft1_sb = ft1pool.tile([128, 128], F32, tag="FT1S")
                nc.scalar.copy(
                    ft1_sb[0:nr1, :], ps[0:nr1, 0, 4 * L : 4 * L + 128]
                )
                ft2_sb = ft2pool.tile([64, 128], BF16, tag="FT2S")
                nc.scalar.copy(
                    ft2_sb[0:nr2, :], ps[0:nr2, 1, 4 * L : 4 * L + 128]
                )
                sup_tiles[s] = (ft1_sb, ft2_sb, nr1, nr2)

            boot_ps = mmpsp.tile([128, TPG, 512], F32, tag="P")
            emit_transposes(0, boot_ps)
            emit_dz_chunks(DZC)
            wq_ps = None
            n_fl = (ntiles + OFL - 1) // OFL
            for fb in range(n_fl):
                nt_f = min(OFL, ntiles - fb * OFL)
                o_t = opool.tile([128, OFL * OUT_DIM], F16, tag="O")
                off = 0
                while off < nt_f:
                    tpg = min(TPG, nt_f - off)
                    j0 = fb * OFL + off
                    s = j0 // SUP
                    sj0 = j0 % SUP
                    ft1_sb, ft2_sb, nr1, nr2 = sup_tiles[s]
                    if sj0 == 0:
                        # first group of a super: wq matmul for all its tiles
                        ns = min(SUP, ntiles - s * SUP)
                        wq_ps = wqpsp.tile([128, 512], F32, tag="WQ")
                        nc.tensor.matmul(
                            wq_ps[:, 0 : ns * L],
                            ft1_sb[0:nr1, 0:128],
                            g1_sb[0:nr1, 0 : ns * L],
                            start=True,
                            stop=True,
                        )
                    psum = mmpsp.tile([128, TPG, 512], F32, tag="P")
                    for jj in range(tpg):
                        j = j0 + jj
                        sj = j % SUP
                        nc.tensor.matmul(
                            psum[:, jj, 0 : 4 * L],
                            ft2_sb[0:nr2, 0:128],
                            g2_sb[0:nr2, sj * 4 * L : (sj + 1) * 4 * L],
                            start=True,
                            stop=True,
                        )
                    if sj0 == 2 and s + 1 < n_sup and (s + 1) not in sup_tiles:
                        emit_transposes(s + 1, psum)
                        emit_dz_chunks(j0 + tpg + DZC)
                    w_t = wpool.tile([128, TPG, L], F32, tag="W")
                    nc.scalar.activation(
                        w_t[:, 0:tpg, :],
                        wq_ps[:, sj0 * L : (sj0 + tpg) * L].rearrange(
                            "p (t l) -> p t l", l=L
                        ),
                        AF.Exp,
                        scale=-0.5,
                    )
                    o4 = o_t[:, off * OUT_DIM : (off + tpg) * OUT_DIM].rearrange(
                        "p (t b l) -> p t b l", b=5, l=L
                    )
                    enc_t = psum[:, 0:tpg, 0 : 4 * L].rearrange(
                        "p t (b l) -> p t b l", l=L
                    )
                    nc.vector.tensor_tensor(
                        o4[:, :, 1:5, :],
                        enc_t,
                        _bcast_block(w_t[:, 0:tpg, :], 4, L),
                        OP.mult,
                    )
                    dz_eng.tensor_tensor(
                        o4[:, :, 0, :],
                        dz_all[:, j0 : j0 + tpg, :],
                        w_t[:, 0:tpg, :],
                        OP.mult,
                    )
                    off += tpg
                nc.sync.dma_start(
                    out_rows[
                        :, fb * OFL * OUT_DIM : (fb * OFL + nt_f) * OUT_DIM
                    ],
                    o_t[:, 0 : nt_f * OUT_DIM],
                )
    nc.compile()
    return nc


def host_params(l, r, scaling, rotation):
    """Zero-row-batched G1 [120, 816] f32, G2 [48, 3264] f32r + t/rz."""
    l = l.astype(np.float64)
    r = r.astype(np.float64)
    scaling = scaling.astype(np.float64)
    rotation = rotation.astype(np.float64)

    rz = r[:3, 2]
    qn = rotation / np.maximum(
        np.linalg.norm(rotation, axis=1, keepdims=True), 1e-12
    )
    w, x, y, z = qn[:, 0], qn[:, 1], qn[:, 2], qn[:, 3]
    R = np.empty((L, 3, 3), np.float64)
    R[:, 0, 0] = 1 - 2 * (y * y + z * z)
    R[:, 0, 1] = 2 * (x * y - w * z)
    R[:, 0, 2] = 2 * (x * z + w * y)
    R[:, 1, 0] = 2 * (x * y + w * z)
    R[:, 1, 1] = 1 - 2 * (x * x + z * z)
    R[:, 1, 2] = 2 * (y * z - w * x)
    R[:, 2, 0] = 2 * (x * z - w * y)
    R[:, 2, 1] = 2 * (y * z + w * x)
    R[:, 2, 2] = 1 - 2 * (x * x + y * y)
    M = R / scaling[:, None, :]
    cov = np.einsum("lij,lkj->lik", M, M)       # [L,3,3]

    b = np.einsum("lij,lj->li", cov, l)         # cov_k @ l_k
    c = np.einsum("li,li->l", l, b)             # l^T cov l
    t = l @ rz

    # G1 core [K1, L]: rows [x (-2b), 1 (c), x^2.. (u)]
    g1c = np.zeros((K1, L), np.float32)
    g1c[0:3] = -2 * b.T
    g1c[3] = c
    g1c[4:7] = np.stack([cov[:, 0, 0], cov[:, 1, 1], cov[:, 2, 2]], 0)
    g1c[7] = 2 * cov[:, 0, 1]
    g1c[8] = 2 * cov[:, 0, 2]
    g1c[9] = 2 * cov[:, 1, 2]
    # G2 core [K2, 4L]: rows [s1,c1,s2,c2]; blocks [sin|cos|sin2|cos2]
    c1, s1 = np.cos(t), np.sin(t)
    c2, s2 = np.cos(2 * t), np.sin(2 * t)
    g2c = np.zeros((K2, 4 * L), np.float32)
    g2c[0, 0:L] = c1
    g2c[1, 0:L] = -s1
    g2c[0, L : 2 * L] = s1
    g2c[1, L : 2 * L] = c1
    g2c[2, 2 * L : 3 * L] = c2
    g2c[3, 2 * L : 3 * L] = -s2
    g2c[2, 3 * L : 4 * L] = s2
    g2c[3, 3 * L : 4 * L] = c2

    G1 = np.zeros((SUP * K1, SUP * L), np.float32)
    G2 = np.zeros((SUP * K2, SUP * 4 * L), np.float32)
    for j in range(SUP):
        G1[j * K1 : (j + 1) * K1, j * L : (j + 1) * L] = g1c
        G2[j * K2 : (j + 1) * K2, j * 4 * L : (j + 1) * 4 * L] = g2c

    import ml_dtypes

    G2 = G2.astype(ml_dtypes.bfloat16)
    tt = np.broadcast_to(t.astype(np.float32), (128, L)).copy()
    rzb = np.broadcast_to(rz.astype(np.float32), (128, 3)).copy()
    return G1, G2, tt, rzb


_NC_CACHE = {}


def _get_nc(pool_dz=True):
    key = bool(pool_dz)
    if key not in _NC_CACHE:
        _NC_CACHE[key] = build_nc(pool_dz=key)
    return _NC_CACHE[key]


def run(inputs, pool_dz=True, trace=False, **_kw):
    x = inputs["x"]
    G1, G2, tt, rzb = host_params(
        inputs["l"], inputs["r"], inputs["scaling"], inputs["rotation"]
    )
    xpad = np.zeros((NCORES * NPAD, 3), np.float32)
    xpad[:N] = x
    shards = xpad.reshape(NCORES, NPAD, 3)
    in_maps = []
    for i in range(NCORES):
        m = {
            "x": np.ascontiguousarray(shards[i]),
            "g1": G1,
            "g2": G2,
            "tt": tt,
            "rzb": rzb,
        }
        in_maps.append(m)
    nc = _get_nc(pool_dz)
    res = run_bass_kernel_spmd(nc, in_maps, list(range(NCORES)), trace=trace)
    out = np.concatenate([r["out"] for r in res.results], axis=0)[:N]
    return np.ascontiguousarray(out.astype(np.float32)), res


def kernel(**inputs):
    out, _ = run(inputs)
    return out
